# revision 20
# baseline (speedup 1.0000x reference)
"""Trainium2 Bass kernel for a pre-norm transformer block (causal MHA + FFN).

Sharding: pure data-parallel over batch B=128 across 8 NeuronCores
(16 batches/core). No collectives.

v3 layout (per core, 4096 tokens as 8 supertiles of 512 tokens = 2 batches):
  - All TensorE operands bf16; PSUM accumulation and residual adds fp32.
  - LayerNorm token-major via bn_stats; rstd = exp(-0.5*ln(var+eps)) so all
    ACT functions live in one table set (loaded once).
  - Feature-major operands produced by PE transposes; the 3 chunk transposes
    of each token tile land in one PSUM bank and move with a single batched
    [128,384] DVE copy.
  - Attention per batch: transposed scores [s,t] per head, exp on ScalarE,
    causal mask on gpsimd, attn+rowsum via 65-wide augmented V slices,
    normalization as a broadcast multiply during PSUM->SBUF.
  - Pipeline: x(g+1) DMA fires at the top of iteration g; LN1(g+1) runs
    inside iteration g's attention window with its PE transposes interleaved
    between attnV heads (behind ff1 filler) so the PE queue never holds a
    transpose that waits on an unfinished DVE chain.  FF2(g-1) is emitted
    before LN2(g)'s transposes; LN2's DVE chain runs under FF2's matmuls.
  - No DMAs for constants: identity is memset+affine_select; bias loads are
    only emitted when biases are nonzero (the zero-bias build skips them).
"""

import sys

for _p in ("/opt/trn_rl_repo",):
    if _p not in sys.path:
        sys.path.append(_p)

import numpy as np
import ml_dtypes

import concourse.bass as bass
import concourse.mybir as mybir
import concourse.tile as tile
from concourse import bacc
from concourse.bass_utils import run_bass_kernel_spmd
# NOTE: walrus's --enable-ldw-opt pass is unusable here: bass's tile
# legalization always emits standalone InstLdweights for non-f32 matmuls,
# and walrus's codegen rejects standalone Ldweights when the pass is on.

B, T, C, H, HS = 128, 256, 384, 6, 64
DFF = 4 * C
EPS = 1e-5
NCORES = 8
BL = B // NCORES          # batches per core (16)
NTOK = BL * T             # tokens per core (4096)
P = 128
CK = C // P               # channel chunks (3)
FK = DFF // P             # ffn chunks (12)
ST = 512                  # supertile tokens (2 batches)
NST = NTOK // ST          # supertiles per core (8)
NTT = ST // P             # token tiles per supertile (4)

F32 = mybir.dt.float32
BF16 = mybir.dt.bfloat16
AF = mybir.ActivationFunctionType
ALU = mybir.AluOpType
BF = ml_dtypes.bfloat16


def build_transformer(nc, with_biases=True):
    xs = nc.dram_tensor("xs", [NTOK, C], F32, kind="ExternalInput").ap()
    wq = nc.dram_tensor("wq", [P, CK, C], BF16, kind="ExternalInput").ap()
    wk = nc.dram_tensor("wk", [P, CK, C], BF16, kind="ExternalInput").ap()
    wv = nc.dram_tensor("wv", [P, CK, C], BF16, kind="ExternalInput").ap()
    wo = nc.dram_tensor("wo", [P, CK, C], BF16, kind="ExternalInput").ap()
    w1 = nc.dram_tensor("w1", [P, CK, DFF], BF16, kind="ExternalInput").ap()
    w2 = nc.dram_tensor("w2", [P, FK, C], BF16, kind="ExternalInput").ap()
    if with_biases:
        bq = nc.dram_tensor("bq", [C], F32, kind="ExternalInput").ap()
        bk = nc.dram_tensor("bk", [C], F32, kind="ExternalInput").ap()
        bv = nc.dram_tensor("bv", [C], BF16, kind="ExternalInput").ap()
        bo = nc.dram_tensor("bo", [C], BF16, kind="ExternalInput").ap()
        b1 = nc.dram_tensor("b1", [DFF], F32, kind="ExternalInput").ap()
        b2 = nc.dram_tensor("b2", [C], BF16, kind="ExternalInput").ap()
    out = nc.dram_tensor("out", [NTOK, C], F32, kind="ExternalOutput").ap()

    from contextlib import ExitStack
    with tile.TileContext(nc) as tc, ExitStack() as ctx:
        const = ctx.enter_context(tc.tile_pool(name="const", bufs=1))
        io_pool = ctx.enter_context(tc.tile_pool(name="iov4", bufs=2))
        act_pool = ctx.enter_context(tc.tile_pool(name="act", bufs=2))
        hn_pool = ctx.enter_context(tc.tile_pool(name="hn", bufs=3))
        wei_pool = ctx.enter_context(tc.tile_pool(name="wei", bufs=8))
        small = ctx.enter_context(tc.tile_pool(name="small", bufs=6))
        rc_pool = ctx.enter_context(tc.tile_pool(name="rc", bufs=6))
        ps_tr = ctx.enter_context(tc.tile_pool(name="ps_tr", bufs=1, space="PSUM"))
        ps_mm = ctx.enter_context(tc.tile_pool(name="ps_mm", bufs=3, space="PSUM"))
        ps_big = ctx.enter_context(tc.tile_pool(name="ps_big", bufs=2, space="PSUM"))
        ps_attn = ctx.enter_context(tc.tile_pool(name="ps_attn", bufs=2, space="PSUM"))

        # ---- x(0) per-token-tile DMAs go first so nothing delays LN1(0) ----
        xa_tiles = {}
        xa0 = io_pool.tile([P, NTT, C], F32, tag="xa", name="xa0")
        for tt in range(NTT):
            nc.sync.dma_start(out=xa0[:, tt, :], in_=xs[tt * P:(tt + 1) * P, :])
        xa_tiles[0] = xa0

        # ---- weight DMAs (after the x triggers; split across both queues) ----
        wq_sb = const.tile([P, CK, C], BF16)
        wk_sb = const.tile([P, CK, C], BF16)
        wv_sb = const.tile([P, CK, C], BF16)
        wo_sb = const.tile([P, CK, C], BF16)
        w1_sb = const.tile([P, CK, DFF], BF16)
        w2_sb = const.tile([P, FK, C], BF16)
        # early-needed weights on the scalar queue (Q10) so the x tiles own
        # Q1 during the pipeline fill; late-needed wo/w2 follow x on Q1
        nc.scalar.dma_start(out=wq_sb, in_=wq)
        nc.scalar.dma_start(out=wk_sb, in_=wk)
        nc.scalar.dma_start(out=wv_sb, in_=wv)
        nc.scalar.dma_start(out=w1_sb, in_=w1)
        nc.sync.dma_start(out=wo_sb, in_=wo)
        nc.sync.dma_start(out=w2_sb, in_=w2)

        # ---- constants without DMA ----
        identity = const.tile([P, P], BF16)
        nc.vector.memset(identity, 1.0)
        nc.gpsimd.affine_select(
            out=identity, in_=identity, compare_op=ALU.is_equal, fill=0.0,
            base=0, pattern=[[1, P]], channel_multiplier=-1)
        eps_tile = const.tile([P, 1], F32)
        nc.vector.memset(eps_tile, EPS)
        ones1 = const.tile([1, P], BF16)
        nc.vector.memset(ones1, 1.0)

        if with_biases:
            bq_sb = const.tile([P, CK], F32)
            nc.sync.dma_start(out=bq_sb, in_=bq.rearrange("(k p) -> p k", p=P))
            bk_sb = const.tile([P, CK], F32)
            nc.sync.dma_start(out=bk_sb, in_=bk.rearrange("(k p) -> p k", p=P))
            b1_sb = const.tile([P, FK], F32)
            nc.sync.dma_start(out=b1_sb, in_=b1.rearrange("(f p) -> p f", p=P))
            bv_row = const.tile([1, C], BF16)
            nc.sync.dma_start(out=bv_row, in_=bv.rearrange("(a d) -> a d", a=1))
            bo_row = const.tile([1, C], BF16)
            nc.sync.dma_start(out=bo_row, in_=bo.rearrange("(a d) -> a d", a=1))
            b2_row = const.tile([1, C], BF16)
            nc.sync.dma_start(out=b2_row, in_=b2.rearrange("(a d) -> a d", a=1))

        # ------------------------------------------------------------------
        # emit helpers
        # ------------------------------------------------------------------
        def emit_xa_dma(g):
            """Trigger the x DMA for supertile g (two halves)."""
            t0 = g * ST
            xa = io_pool.tile([P, NTT, C], F32, tag="xa", name=f"xa{g}")
            for hf in range(2):
                nc.sync.dma_start(
                    out=xa[:, 2 * hf:2 * hf + 2, :],
                    in_=xs[t0 + hf * 2 * P:t0 + (hf + 1) * 2 * P, :].rearrange(
                        "(tt p) c -> p tt c", p=P))
            xa_tiles[g] = xa
            return xa

        def emit_ln_dve(xa_t, tt, hn_tag):
            """Token-major LN stats + normalize for one token tile (DVE+ACT)."""
            xt = xa_t[:, tt, :]
            stats = small.tile([P, 6], F32, tag="stats", name=f"st_{hn_tag}{tt}")
            nc.vector.bn_stats(out=stats, in_=xt)
            mv = small.tile([P, 2], F32, tag="mv", name=f"mv_{hn_tag}{tt}")
            nc.vector.bn_aggr(out=mv, in_=stats)
            lnv = small.tile([P, 1], F32, tag="lnv", name=f"ln_{hn_tag}{tt}")
            nc.scalar.activation(out=lnv, in_=mv[:, 1:2], func=AF.Ln, bias=eps_tile)
            rstd = small.tile([P, 1], F32, tag="rstd", name=f"rs_{hn_tag}{tt}")
            nc.scalar.activation(out=rstd, in_=lnv, func=AF.Exp, scale=-0.5)
            hn = hn_pool.tile([P, C], BF16, tag="hn", bufs=6, name=f"hn_{hn_tag}{tt}")
            nc.vector.tensor_scalar(
                out=hn, in0=xt, scalar1=mv[:, 0:1], scalar2=rstd,
                op0=ALU.subtract, op1=ALU.mult)
            return hn

        def emit_tr(src_tm, dst_T, tt, copy_engine=None):
            """PE-transpose one token tile (3 chunks into one PSUM bank) and
            move it feature-major with a single batched copy."""
            pst = ps_tr.tile([P, C], BF16, tag="tr", name=f"tr{tt}")
            for k in range(CK):
                nc.tensor.transpose(
                    pst[:, k * P:(k + 1) * P], src_tm[:, k * P:(k + 1) * P],
                    identity)
            if copy_engine == "scalar":
                nc.scalar.activation(
                    out=dst_T[:, :, tt * P:(tt + 1) * P],
                    in_=pst.rearrange("p (k q) -> p k q", q=P),
                    func=AF.Identity)
            else:
                nc.vector.tensor_copy(
                    out=dst_T[:, :, tt * P:(tt + 1) * P],
                    in_=pst.rearrange("p (k q) -> p k q", q=P))

        def emit_tr_xbar(src_tm, dst_T, tt):
            """Feature-major transpose via the DMA crossbar: no PE, no PSUM,
            no copy — one descriptor-generation slot on the sync queue."""
            nc.sync.dma_start_transpose(
                out=dst_T[:, :, tt * P:(tt + 1) * P], in_=src_tm)

        def emit_qkv(g, h1T, half=None):
            """QKV projections.  half=None: full N=512; half=0/1: N=256."""
            co = 0 if half is None else half * 256
            n = ST if half is None else 256
            QT = qkt_tiles.setdefault(
                (g, "q"), act_pool.tile([P, CK, ST], BF16, tag="QT", name=f"QT{g}"))
            KT = qkt_tiles.setdefault(
                (g, "k"), act_pool.tile([P, CK, ST], BF16, tag="KT", name=f"KT{g}"))
            for m in range(CK):
                psq = ps_mm.tile([P, n], F32, tag="mm", name=f"psq{g}{m}")
                for k in range(CK):
                    nc.tensor.matmul(
                        psq, wq_sb[:, k, m * P:(m + 1) * P], h1T[:, k, co:co + n],
                        start=(k == 0), stop=(k == CK - 1))
                nc.scalar.activation(
                    out=QT[:, m, co:co + n], in_=psq, func=AF.Identity,
                    bias=(bq_sb[:, m:m + 1] if with_biases else 0.0))
                psk = ps_mm.tile([P, n], F32, tag="mm", name=f"psk{g}{m}")
                for k in range(CK):
                    nc.tensor.matmul(
                        psk, wk_sb[:, k, m * P:(m + 1) * P], h1T[:, k, co:co + n],
                        start=(k == 0), stop=(k == CK - 1))
                nc.scalar.activation(
                    out=KT[:, m, co:co + n], in_=psk, func=AF.Identity,
                    bias=(bk_sb[:, m:m + 1] if with_biases else 0.0))
            return QT, KT

        def emit_v(g, h1T, tts):
            """V projection for the given token tiles; writes the 65-wide
            augmented token-major Vtm (ones column feeds softmax row sums)."""
            Vtm = vtm_tiles.setdefault(
                g, act_pool.tile([P, NTT, H * 65], BF16, tag="Vtm", name=f"Vtm{g}"))
            for tt in tts:
                psv = ps_big.tile([P, C], F32, tag="big", name=f"psv{g}{tt}")
                for k in range(CK):
                    nc.tensor.matmul(
                        psv, h1T[:, k, tt * P:(tt + 1) * P], wv_sb[:, k, :],
                        start=(k == 0), stop=(not with_biases and k == CK - 1))
                if with_biases:
                    nc.tensor.matmul(psv, ones1, bv_row, start=False, stop=True)
                vview = Vtm[:, tt, :].rearrange("p (h e) -> p h e", e=65)
                nc.vector.tensor_copy(
                    out=vview[:, :, 0:HS],
                    in_=psv.rearrange("p (h e) -> p h e", e=HS))
                nc.gpsimd.memset(vview[:, :, HS:65], 1.0)
            return Vtm

        def emit_scores(g, QT, KT, b2):
            """Blocked causal scores + exp + mask for one batch; returns the
            per-head transposed probability tiles."""
            co = b2 * T
            weiTs = [
                wei_pool.tile([P, 3 * P], BF16, tag="weiT", name=f"w{g}_{b2}_{h}")
                for h in range(H)
            ]
            for hp in range(H // 2):
                h0, h1 = 2 * hp, 2 * hp + 1
                q0 = QT[0:HS, hp, co:co + T]
                k0 = KT[0:HS, hp, co:co + T]
                q1 = QT[HS:2 * HS, hp, co:co + T]
                k1 = KT[HS:2 * HS, hp, co:co + T]
                ps0 = ps_mm.tile([P, ST], F32, tag="mm", name=f"s{g}{b2}{hp}0")
                ps1 = ps_mm.tile([P, ST], F32, tag="mm", name=f"s{g}{b2}{hp}1")
                nc.tensor.matmul(ps0[:, 0:T], k0[:, 0:P], q0, start=True, stop=True)
                nc.tensor.matmul(ps1[:, 0:T], k1[:, 0:P], q1, start=True, stop=True)
                nc.tensor.matmul(ps0[:, T:T + P], k0[:, P:], q0[:, P:],
                                 start=True, stop=True)
                nc.tensor.matmul(ps1[:, T:T + P], k1[:, P:], q1[:, P:],
                                 start=True, stop=True)
                for h, pss in ((h0, ps0), (h1, ps1)):
                    weiT = weiTs[h]
                    nc.scalar.activation(
                        out=weiT, in_=pss[:, 0:3 * P], func=AF.Exp,
                        scale=HS ** -0.5)
                    nc.gpsimd.affine_select(
                        out=weiT[:, 0:P], in_=weiT[:, 0:P],
                        compare_op=ALU.is_ge, fill=0.0, base=0,
                        pattern=[[1, P]], channel_multiplier=-1)
                    nc.gpsimd.affine_select(
                        out=weiT[:, 2 * P:], in_=weiT[:, 2 * P:],
                        compare_op=ALU.is_ge, fill=0.0, base=0,
                        pattern=[[1, P]], channel_multiplier=-1)
            return weiTs

        def emit_attnv_head(weiT, Vtm, b2, attn_ps, h):
            vo = b2 * 2
            for tt in range(2):
                dst = attn_ps[tt][:, h * 65:(h + 1) * 65]
                if tt == 0:
                    nc.tensor.matmul(
                        dst, weiT[:, 0:P], Vtm[:, vo, h * 65:(h + 1) * 65],
                        start=True, stop=True)
                else:
                    nc.tensor.matmul(
                        dst, weiT[:, P:2 * P], Vtm[:, vo, h * 65:(h + 1) * 65],
                        start=True, stop=False)
                    nc.tensor.matmul(
                        dst, weiT[:, 2 * P:], Vtm[:, vo + 1, h * 65:(h + 1) * 65],
                        start=False, stop=True)

        def emit_norm(g, attn_ps, attn_sb, b2):
            """One reciprocal over the interleaved row sums, then normalize
            all heads during the PSUM->SBUF move (broadcast multiply)."""
            vo = b2 * 2
            for tt in range(2):
                aview = attn_ps[tt].rearrange("p (h e) -> p h e", e=65)
                rc6 = rc_pool.tile([P, H], F32, tag="rc", name=f"rc{g}{b2}{tt}")
                nc.vector.reciprocal(out=rc6, in_=aview[:, :, HS])
                rc_b = bass.AP(
                    tensor=rc6.tensor, offset=rc6.offset,
                    ap=[rc6.ap[0], rc6.ap[1], [0, HS]])
                nc.vector.tensor_tensor(
                    out=attn_sb[:, vo + tt, :].rearrange("p (h e) -> p h e", e=HS),
                    in0=aview[:, :, 0:HS], in1=rc_b, op=ALU.mult)

        def emit_wo_tt(g, attn_T, xa_t, xmid, tt):
            pso = ps_big.tile([P, C], F32, tag="big", name=f"pso{g}{tt}")
            for k in range(CK):
                nc.tensor.matmul(
                    pso, attn_T[:, k, tt * P:(tt + 1) * P], wo_sb[:, k, :],
                    start=(k == 0), stop=(not with_biases and k == CK - 1))
            if with_biases:
                nc.tensor.matmul(pso, ones1, bo_row, start=False, stop=True)
            nc.vector.tensor_add(out=xmid[:, tt, :], in0=xa_t[:, tt, :], in1=pso)

        def emit_ff1_third(g, h2T, ff1T, fs):
            for f in fs:
                psf = ps_mm.tile([P, ST], F32, tag="mm", name=f"psf{g}{f}")
                for k in range(CK):
                    nc.tensor.matmul(
                        psf, w1_sb[:, k, f * P:(f + 1) * P], h2T[:, k, :],
                        start=(k == 0), stop=(k == CK - 1))
                nc.scalar.activation(
                    out=ff1T[:, f, :], in_=psf, func=AF.Relu,
                    bias=(b1_sb[:, f:f + 1] if with_biases else 0.0))

        def emit_ff2(g, ff1T, xmid, t0):
            for tt in range(NTT):
                ps2 = ps_big.tile([P, C], F32, tag="big", name=f"ps2{g}{tt}")
                for f in range(FK):
                    nc.tensor.matmul(
                        ps2, ff1T[:, f, tt * P:(tt + 1) * P], w2_sb[:, f, :],
                        start=(f == 0), stop=(not with_biases and f == FK - 1))
                if with_biases:
                    nc.tensor.matmul(ps2, ones1, b2_row, start=False, stop=True)
                yt = io_pool.tile([P, C], F32, tag="yt", name=f"yt{g}{tt}")
                nc.vector.tensor_add(out=yt, in0=xmid[:, tt, :], in1=ps2)
                nc.sync.dma_start(
                    out=out[t0 + tt * P: t0 + (tt + 1) * P, :], in_=yt)

        # ------------------------------------------------------------------
        # supertile 0 prologue: interleave LN1(0) with QKV halves so the PE
        # starts as soon as the first token tiles land
        # ------------------------------------------------------------------
        qkt_tiles = {}
        vtm_tiles = {}
        h1T0 = act_pool.tile([P, CK, ST], BF16, tag="h1T", name="h1T0")
        for tt in (0, 1):
            hn = emit_ln_dve(xa0, tt, "a0")
            emit_tr_xbar(hn, h1T0, tt)
        emit_qkv(0, h1T0, half=0)
        emit_v(0, h1T0, (0, 1))
        for tt in (2, 3):
            hn = emit_ln_dve(xa0, tt, "a0")
            emit_tr_xbar(hn, h1T0, tt)
        emit_qkv(0, h1T0, half=1)
        emit_v(0, h1T0, (2, 3))
        h1T_cur = h1T0

        pend = None            # (h2T, xmid, t0) of supertile g-1
        for g in range(NST):
            t0 = g * ST
            xa_t = xa_tiles[g]

            if g + 1 < NST:
                xa_nxt = emit_xa_dma(g + 1)
                h1T_nxt = act_pool.tile(
                    [P, CK, ST], BF16, tag="h1T", name=f"h1T{g + 1}")
            if g > 0:
                # QKV/V for this supertile (prologue already did g=0)
                emit_qkv(g, h1T_cur)
                emit_v(g, h1T_cur, range(NTT))
            QT, KT = qkt_tiles[(g, "q")], qkt_tiles[(g, "k")]
            Vtm = vtm_tiles[g]

            if pend is not None:
                ff1T = act_pool.tile([P, FK, ST], BF16, tag="ff1T",
                                     name=f"ff1T{g - 1}")

            # ---- attention (2 batches) with ff1 filler, LN1(g+1) transposes
            # and the first attn transposes threaded between attnV heads ----
            attn_sb = hn_pool.tile([P, NTT, C], BF16, tag="attn_sb",
                                   name=f"asb{g}")
            attn_T = act_pool.tile([P, CK, ST], BF16, tag="attnT", name=f"aT{g}")
            for b2 in range(2):
                hns = {}
                if g + 1 < NST:
                    for tt in (2 * b2, 2 * b2 + 1):
                        hns[tt] = emit_ln_dve(xa_nxt, tt, f"a{g + 1}")
                weiTs = emit_scores(g, QT, KT, b2)
                attn_ps = [
                    ps_attn.tile([P, H * 65], F32, tag="attn",
                                 name=f"aps{g}_{b2}_{tt}")
                    for tt in range(2)
                ]
                if pend is not None:
                    emit_ff1_third(g, pend[0], ff1T, range(6 * b2, 6 * b2 + 3))
                for h in range(H):
                    emit_attnv_head(weiTs[h], Vtm, b2, attn_ps, h)
                    if h == 2:
                        if g + 1 < NST:
                            emit_tr_xbar(hns[2 * b2], h1T_nxt, 2 * b2)
                        if pend is not None:
                            emit_ff1_third(g, pend[0], ff1T,
                                           range(6 * b2 + 3, 6 * b2 + 6))
                        if b2 == 1:
                            emit_tr(attn_sb[:, 0, :], attn_T, 0)
                if g + 1 < NST:
                    emit_tr_xbar(hns[2 * b2 + 1], h1T_nxt, 2 * b2 + 1)
                emit_norm(g, attn_ps, attn_sb, b2)
                if b2 == 1:
                    emit_tr(attn_sb[:, 1, :], attn_T, 1)

            # ---- remaining attn transposes interleaved with Wo + residual ----
            xmid = io_pool.tile([P, NTT, C], F32, tag="xmid", name=f"xm{g}")
            emit_tr(attn_sb[:, 2, :], attn_T, 2)
            emit_wo_tt(g, attn_T, xa_t, xmid, 0)
            emit_tr(attn_sb[:, 3, :], attn_T, 3)
            for tt in range(1, NTT):
                emit_wo_tt(g, attn_T, xa_t, xmid, tt)

            # ---- LN2 DVE chains run under FF2(g-1)'s matmuls; the LN2
            # transposes are emitted after FF2 so they never block the PE ----
            h2T = act_pool.tile([P, CK, ST], BF16, tag="h2T", name=f"h2T{g}")
            hn2 = {tt: emit_ln_dve(xmid, tt, f"m{g}") for tt in range(NTT)}
            if pend is not None:
                emit_ff2(g, ff1T, pend[1], pend[2])
            for tt in range(NTT):
                emit_tr_xbar(hn2[tt], h2T, tt)

            pend = (h2T, xmid, t0)
            if g + 1 < NST:
                h1T_cur = h1T_nxt

        # epilogue: final supertile's FFN
        ff1T = act_pool.tile([P, FK, ST], BF16, tag="ff1T", name="ff1T7")
        emit_ff1_third(NST, pend[0], ff1T, range(0, 6))
        emit_ff1_third(NST, pend[0], ff1T, range(6, 12))
        emit_ff2(NST, ff1T, pend[1], pend[2])
    return nc


_NC_CACHE = {}


class _PinnedActBacc(bacc.Bacc):
    """Pin all ACT functions to the natural_log_exp_and_others table set.

    The kernel only uses Exp, Ln, Relu and Identity, all of which live in
    that one set; the default per-function greedy pick alternates between
    exp/sqrt/log sets and pays ~2.7us per switch. Blanking the other sets
    (indexes preserved) makes the fixpoint choose one set, loaded once.
    """

    def insert_act_table_loads(self):
        import concourse.mybir as _mb
        from concourse.hw_specs import get_activation_tables
        has_activation = any(
            isinstance(i, _mb.InstActivation)
            for b in self.main_func.blocks
            for i in b.instructions
        )
        if not has_activation:
            return
        keep = "natural_log_exp_and_others"
        tables = [
            (k, (v if k == keep else set()))
            for k, v in get_activation_tables(self.m.arch).items()
        ]
        bacc._bass_rust.insert_act_table_loads(self, tables)


def get_nc(with_biases=True):
    key = f"nc_b{int(with_biases)}"
    if key not in _NC_CACHE:
        nc = _PinnedActBacc(
            "TRN2", target_bir_lowering=False, debug=False, num_devices=NCORES)
        build_transformer(nc, with_biases=with_biases)
        nc.compile()
        _NC_CACHE[key] = nc
    return _NC_CACHE[key]


def prep_inputs(x, Wq, Wk, Wv, Wo, bo, W1, b1, W2, b2, g1, be1, g2, be2):
    """Host-side exact folding of LN affine params into weights/biases, plus
    layout packing and bf16 casts."""
    f32 = np.float32
    x = np.asarray(x, f32)
    Wq2 = np.asarray(Wq, f32).transpose(1, 0, 2).reshape(C, C)
    Wk2 = np.asarray(Wk, f32).transpose(1, 0, 2).reshape(C, C)
    Wv2 = np.asarray(Wv, f32).transpose(1, 0, 2).reshape(C, C)
    g1 = np.asarray(g1, f32)
    be1 = np.asarray(be1, f32)
    g2 = np.asarray(g2, f32)
    be2 = np.asarray(be2, f32)
    bf = lambda a: np.ascontiguousarray(np.asarray(a, f32)).astype(BF)
    shared = {
        "wq": bf((g1[:, None] * Wq2).reshape(CK, P, C).transpose(1, 0, 2)),
        "wk": bf((g1[:, None] * Wk2).reshape(CK, P, C).transpose(1, 0, 2)),
        "wv": bf((g1[:, None] * Wv2).reshape(CK, P, C).transpose(1, 0, 2)),
        "wo": bf(np.asarray(Wo, f32).reshape(CK, P, C).transpose(1, 0, 2)),
        "w1": bf((g2[:, None] * np.asarray(W1, f32)).reshape(CK, P, DFF).transpose(1, 0, 2)),
        "w2": bf(np.asarray(W2, f32).reshape(FK, P, C).transpose(1, 0, 2)),
    }
    biases = {
        "bq": np.ascontiguousarray(be1 @ Wq2),
        "bk": np.ascontiguousarray(be1 @ Wk2),
        "bv": bf(be1 @ Wv2),
        "bo": bf(np.asarray(bo, f32)),
        "b1": np.ascontiguousarray(be2 @ np.asarray(W1, f32) + np.asarray(b1, f32)),
        "b2": bf(np.asarray(b2, f32)),
    }
    shards = [
        np.ascontiguousarray(x[i * BL:(i + 1) * BL].reshape(NTOK, C))
        for i in range(NCORES)
    ]
    return shared, biases, shards


def all_biases_zero(biases):
    return all(
        not np.any(np.asarray(biases[k], np.float32))
        for k in ("bq", "bk", "bv", "bo", "b1", "b2"))


def run_on_device(nc, shared, shards, trace=False, **kwargs):
    in_maps = [dict(shared, xs=shards[i]) for i in range(NCORES)]
    return run_bass_kernel_spmd(
        nc, in_maps, core_ids=list(range(NCORES)), trace=trace, **kwargs)


def kernel(**inputs):
    shared, biases, shards = prep_inputs(**inputs)
    wb = not all_biases_zero(biases)
    if wb:
        shared = dict(shared, **biases)
    nc = get_nc(with_biases=wb)
    res = run_on_device(nc, shared, shards, trace=False)
    out = np.concatenate(
        [res.results[i]["out"].reshape(BL, T, C) for i in range(NCORES)], axis=0)
    return out.astype(np.float32)


# revision 21
# speedup vs baseline: 1.1089x; 1.1089x over previous
"""Trainium2 Bass kernel for a pre-norm transformer block (causal MHA + FFN).

Sharding: pure data-parallel over batch B=128 across 8 NeuronCores
(16 batches/core). No collectives.

v3 layout (per core, 4096 tokens as 8 supertiles of 512 tokens = 2 batches):
  - All TensorE operands bf16; PSUM accumulation and residual adds fp32.
  - LayerNorm token-major via bn_stats; rstd = exp(-0.5*ln(var+eps)) so all
    ACT functions live in one table set (loaded once).
  - Feature-major operands produced by PE transposes; the 3 chunk transposes
    of each token tile land in one PSUM bank and move with a single batched
    [128,384] DVE copy.
  - Attention per batch: transposed scores [s,t] per head, exp on ScalarE,
    causal mask on gpsimd, attn+rowsum via 65-wide augmented V slices,
    normalization as a broadcast multiply during PSUM->SBUF.
  - Pipeline: x(g+1) DMA fires at the top of iteration g; LN1(g+1) runs
    inside iteration g's attention window with its PE transposes interleaved
    between attnV heads (behind ff1 filler) so the PE queue never holds a
    transpose that waits on an unfinished DVE chain.  FF2(g-1) is emitted
    before LN2(g)'s transposes; LN2's DVE chain runs under FF2's matmuls.
  - No DMAs for constants: identity is memset+affine_select; bias loads are
    only emitted when biases are nonzero (the zero-bias build skips them).
"""

import sys

for _p in ("/opt/trn_rl_repo",):
    if _p not in sys.path:
        sys.path.append(_p)

import numpy as np
import ml_dtypes

import concourse.bass as bass
import concourse.mybir as mybir
import concourse.tile as tile
from concourse import bacc
from concourse.bass_utils import run_bass_kernel_spmd
# NOTE: walrus's --enable-ldw-opt pass is unusable here: bass's tile
# legalization always emits standalone InstLdweights for non-f32 matmuls,
# and walrus's codegen rejects standalone Ldweights when the pass is on.

B, T, C, H, HS = 128, 256, 384, 6, 64
DFF = 4 * C
EPS = 1e-5
NCORES = 8
BL = B // NCORES          # batches per core (16)
NTOK = BL * T             # tokens per core (4096)
P = 128
CK = C // P               # channel chunks (3)
FK = DFF // P             # ffn chunks (12)
ST = 512                  # supertile tokens (2 batches)
NST = NTOK // ST          # supertiles per core (8)
NTT = ST // P             # token tiles per supertile (4)

F32 = mybir.dt.float32
BF16 = mybir.dt.bfloat16
AF = mybir.ActivationFunctionType
ALU = mybir.AluOpType
BF = ml_dtypes.bfloat16


def build_transformer(nc, with_biases=True):
    xs = nc.dram_tensor("xs", [NTOK, C], F32, kind="ExternalInput").ap()
    wq = nc.dram_tensor("wq", [P, CK, C], BF16, kind="ExternalInput").ap()
    wk = nc.dram_tensor("wk", [P, CK, C], BF16, kind="ExternalInput").ap()
    wv = nc.dram_tensor("wv", [P, CK, C], BF16, kind="ExternalInput").ap()
    wo = nc.dram_tensor("wo", [P, CK, C], BF16, kind="ExternalInput").ap()
    w1 = nc.dram_tensor("w1", [P, CK, DFF], BF16, kind="ExternalInput").ap()
    w2 = nc.dram_tensor("w2", [P, FK, C], BF16, kind="ExternalInput").ap()
    if with_biases:
        bq = nc.dram_tensor("bq", [C], F32, kind="ExternalInput").ap()
        bk = nc.dram_tensor("bk", [C], F32, kind="ExternalInput").ap()
        bv = nc.dram_tensor("bv", [C], BF16, kind="ExternalInput").ap()
        bo = nc.dram_tensor("bo", [C], BF16, kind="ExternalInput").ap()
        b1 = nc.dram_tensor("b1", [DFF], F32, kind="ExternalInput").ap()
        b2 = nc.dram_tensor("b2", [C], BF16, kind="ExternalInput").ap()
    out = nc.dram_tensor("out", [NTOK, C], F32, kind="ExternalOutput").ap()

    from contextlib import ExitStack
    with tile.TileContext(nc) as tc, ExitStack() as ctx:
        const = ctx.enter_context(tc.tile_pool(name="const", bufs=1))
        io_pool = ctx.enter_context(tc.tile_pool(name="iov4", bufs=2))
        act_pool = ctx.enter_context(tc.tile_pool(name="act", bufs=2))
        hn_pool = ctx.enter_context(tc.tile_pool(name="hn", bufs=3))
        wei_pool = ctx.enter_context(tc.tile_pool(name="wei", bufs=8))
        small = ctx.enter_context(tc.tile_pool(name="small", bufs=6))
        rc_pool = ctx.enter_context(tc.tile_pool(name="rc", bufs=6))
        ps_tr = ctx.enter_context(tc.tile_pool(name="ps_tr", bufs=1, space="PSUM"))
        ps_mm = ctx.enter_context(tc.tile_pool(name="ps_mm", bufs=3, space="PSUM"))
        ps_big = ctx.enter_context(tc.tile_pool(name="ps_big", bufs=2, space="PSUM"))
        ps_attn = ctx.enter_context(tc.tile_pool(name="ps_attn", bufs=2, space="PSUM"))

        # ---- x(0) per-token-tile DMAs go first so nothing delays LN1(0) ----
        xa_tiles = {}
        xa0 = io_pool.tile([P, NTT, C], F32, tag="xa", name="xa0")
        for tt in range(NTT):
            nc.sync.dma_start(out=xa0[:, tt, :], in_=xs[tt * P:(tt + 1) * P, :])
        xa_tiles[0] = xa0

        # ---- weight DMAs (after the x triggers; split across both queues) ----
        wq_sb = const.tile([P, CK, C], BF16)
        wk_sb = const.tile([P, CK, C], BF16)
        wv_sb = const.tile([P, CK, C], BF16)
        wo_sb = const.tile([P, CK, C], BF16)
        w1_sb = const.tile([P, CK, DFF], BF16)
        w2_sb = const.tile([P, FK, C], BF16)
        # early-needed weights on the scalar queue (Q10) so the x tiles own
        # Q1 during the pipeline fill; late-needed wo/w2 follow x on Q1
        nc.scalar.dma_start(out=wq_sb, in_=wq)
        nc.scalar.dma_start(out=wk_sb, in_=wk)
        nc.scalar.dma_start(out=wv_sb, in_=wv)
        nc.scalar.dma_start(out=w1_sb, in_=w1)
        nc.sync.dma_start(out=wo_sb, in_=wo)
        nc.sync.dma_start(out=w2_sb, in_=w2)

        # ---- constants without DMA ----
        identity = const.tile([P, P], BF16)
        nc.vector.memset(identity, 1.0)
        nc.gpsimd.affine_select(
            out=identity, in_=identity, compare_op=ALU.is_equal, fill=0.0,
            base=0, pattern=[[1, P]], channel_multiplier=-1)
        eps_tile = const.tile([P, 1], F32)
        nc.vector.memset(eps_tile, EPS)
        ones1 = const.tile([1, P], BF16)
        nc.vector.memset(ones1, 1.0)

        if with_biases:
            bq_sb = const.tile([P, CK], F32)
            nc.sync.dma_start(out=bq_sb, in_=bq.rearrange("(k p) -> p k", p=P))
            bk_sb = const.tile([P, CK], F32)
            nc.sync.dma_start(out=bk_sb, in_=bk.rearrange("(k p) -> p k", p=P))
            b1_sb = const.tile([P, FK], F32)
            nc.sync.dma_start(out=b1_sb, in_=b1.rearrange("(f p) -> p f", p=P))
            bv_row = const.tile([1, C], BF16)
            nc.sync.dma_start(out=bv_row, in_=bv.rearrange("(a d) -> a d", a=1))
            bo_row = const.tile([1, C], BF16)
            nc.sync.dma_start(out=bo_row, in_=bo.rearrange("(a d) -> a d", a=1))
            b2_row = const.tile([1, C], BF16)
            nc.sync.dma_start(out=b2_row, in_=b2.rearrange("(a d) -> a d", a=1))

        # ------------------------------------------------------------------
        # emit helpers
        # ------------------------------------------------------------------
        def emit_xa_dma(g):
            """Trigger the x DMA for supertile g (two halves)."""
            t0 = g * ST
            xa = io_pool.tile([P, NTT, C], F32, tag="xa", name=f"xa{g}")
            for hf in range(2):
                nc.sync.dma_start(
                    out=xa[:, 2 * hf:2 * hf + 2, :],
                    in_=xs[t0 + hf * 2 * P:t0 + (hf + 1) * 2 * P, :].rearrange(
                        "(tt p) c -> p tt c", p=P))
            xa_tiles[g] = xa
            return xa

        def emit_ln_dve(xa_t, tt, hn_tag):
            """Token-major LN stats + normalize for one token tile (DVE+ACT)."""
            xt = xa_t[:, tt, :]
            stats = small.tile([P, 6], F32, tag="stats", name=f"st_{hn_tag}{tt}")
            nc.vector.bn_stats(out=stats, in_=xt)
            mv = small.tile([P, 2], F32, tag="mv", name=f"mv_{hn_tag}{tt}")
            nc.vector.bn_aggr(out=mv, in_=stats)
            lnv = small.tile([P, 1], F32, tag="lnv", name=f"ln_{hn_tag}{tt}")
            nc.scalar.activation(out=lnv, in_=mv[:, 1:2], func=AF.Ln, bias=eps_tile)
            rstd = small.tile([P, 1], F32, tag="rstd", name=f"rs_{hn_tag}{tt}")
            nc.scalar.activation(out=rstd, in_=lnv, func=AF.Exp, scale=-0.5)
            hn = hn_pool.tile([P, C], BF16, tag="hn", bufs=6, name=f"hn_{hn_tag}{tt}")
            nc.vector.tensor_scalar(
                out=hn, in0=xt, scalar1=mv[:, 0:1], scalar2=rstd,
                op0=ALU.subtract, op1=ALU.mult)
            return hn

        def emit_tr(src_tm, dst_T, tt, copy_engine=None):
            """PE-transpose one token tile (3 chunks into one PSUM bank) and
            move it feature-major with a single batched copy."""
            pst = ps_tr.tile([P, C], BF16, tag="tr", name=f"tr{tt}")
            for k in range(CK):
                nc.tensor.transpose(
                    pst[:, k * P:(k + 1) * P], src_tm[:, k * P:(k + 1) * P],
                    identity)
            if copy_engine == "scalar":
                nc.scalar.activation(
                    out=dst_T[:, :, tt * P:(tt + 1) * P],
                    in_=pst.rearrange("p (k q) -> p k q", q=P),
                    func=AF.Identity)
            else:
                nc.vector.tensor_copy(
                    out=dst_T[:, :, tt * P:(tt + 1) * P],
                    in_=pst.rearrange("p (k q) -> p k q", q=P))

        def emit_tr_xbar(src_tm, dst_T, tt):
            """Feature-major transpose via the DMA crossbar: no PE, no PSUM,
            no copy — one descriptor-generation slot on the sync queue."""
            nc.sync.dma_start_transpose(
                out=dst_T[:, :, tt * P:(tt + 1) * P], in_=src_tm)

        def emit_qkv(g, h1T, half=None):
            """QKV projections.  half=None: full N=512; half=0/1: N=256."""
            co = 0 if half is None else half * 256
            n = ST if half is None else 256
            QT = qkt_tiles.setdefault(
                (g, "q"), act_pool.tile([P, CK, ST], BF16, tag="QT", name=f"QT{g}"))
            KT = qkt_tiles.setdefault(
                (g, "k"), act_pool.tile([P, CK, ST], BF16, tag="KT", name=f"KT{g}"))
            for m in range(CK):
                psq = ps_mm.tile([P, n], F32, tag="mm", name=f"psq{g}{m}")
                for k in range(CK):
                    nc.tensor.matmul(
                        psq, wq_sb[:, k, m * P:(m + 1) * P], h1T[:, k, co:co + n],
                        start=(k == 0), stop=(k == CK - 1))
                nc.scalar.activation(
                    out=QT[:, m, co:co + n], in_=psq, func=AF.Identity,
                    bias=(bq_sb[:, m:m + 1] if with_biases else 0.0))
                psk = ps_mm.tile([P, n], F32, tag="mm", name=f"psk{g}{m}")
                for k in range(CK):
                    nc.tensor.matmul(
                        psk, wk_sb[:, k, m * P:(m + 1) * P], h1T[:, k, co:co + n],
                        start=(k == 0), stop=(k == CK - 1))
                nc.scalar.activation(
                    out=KT[:, m, co:co + n], in_=psk, func=AF.Identity,
                    bias=(bk_sb[:, m:m + 1] if with_biases else 0.0))
            return QT, KT

        def emit_v(g, h1T, tts):
            """V projection for the given token tiles; writes the 65-wide
            augmented token-major Vtm (ones column feeds softmax row sums)."""
            Vtm = vtm_tiles.setdefault(
                g, act_pool.tile([P, NTT, H * 65], BF16, tag="Vtm", name=f"Vtm{g}"))
            for tt in tts:
                psv = ps_big.tile([P, C], F32, tag="big", name=f"psv{g}{tt}")
                for k in range(CK):
                    nc.tensor.matmul(
                        psv, h1T[:, k, tt * P:(tt + 1) * P], wv_sb[:, k, :],
                        start=(k == 0), stop=(not with_biases and k == CK - 1))
                if with_biases:
                    nc.tensor.matmul(psv, ones1, bv_row, start=False, stop=True)
                vview = Vtm[:, tt, :].rearrange("p (h e) -> p h e", e=65)
                nc.vector.tensor_copy(
                    out=vview[:, :, 0:HS],
                    in_=psv.rearrange("p (h e) -> p h e", e=HS))
                nc.gpsimd.memset(vview[:, :, HS:65], 1.0)
            return Vtm

        def emit_scores(g, QT, KT, b2):
            """Blocked causal scores + exp + mask for one batch; returns the
            per-head transposed probability tiles."""
            co = b2 * T
            weiTs = [
                wei_pool.tile([P, 3 * P], BF16, tag="weiT", name=f"w{g}_{b2}_{h}")
                for h in range(H)
            ]
            for hp in range(H // 2):
                h0, h1 = 2 * hp, 2 * hp + 1
                q0 = QT[0:HS, hp, co:co + T]
                k0 = KT[0:HS, hp, co:co + T]
                q1 = QT[HS:2 * HS, hp, co:co + T]
                k1 = KT[HS:2 * HS, hp, co:co + T]
                ps0 = ps_mm.tile([P, ST], F32, tag="mm", name=f"s{g}{b2}{hp}0")
                ps1 = ps_mm.tile([P, ST], F32, tag="mm", name=f"s{g}{b2}{hp}1")
                nc.tensor.matmul(ps0[:, 0:T], k0[:, 0:P], q0, start=True, stop=True)
                nc.tensor.matmul(ps1[:, 0:T], k1[:, 0:P], q1, start=True, stop=True)
                nc.tensor.matmul(ps0[:, T:T + P], k0[:, P:], q0[:, P:],
                                 start=True, stop=True)
                nc.tensor.matmul(ps1[:, T:T + P], k1[:, P:], q1[:, P:],
                                 start=True, stop=True)
                for h, pss in ((h0, ps0), (h1, ps1)):
                    weiT = weiTs[h]
                    nc.scalar.activation(
                        out=weiT, in_=pss[:, 0:3 * P], func=AF.Exp,
                        scale=HS ** -0.5)
                    nc.gpsimd.affine_select(
                        out=weiT[:, 0:P], in_=weiT[:, 0:P],
                        compare_op=ALU.is_ge, fill=0.0, base=0,
                        pattern=[[1, P]], channel_multiplier=-1)
                    nc.gpsimd.affine_select(
                        out=weiT[:, 2 * P:], in_=weiT[:, 2 * P:],
                        compare_op=ALU.is_ge, fill=0.0, base=0,
                        pattern=[[1, P]], channel_multiplier=-1)
            return weiTs

        def emit_attnv_head(weiT, Vtm, b2, attn_ps, h):
            vo = b2 * 2
            for tt in range(2):
                dst = attn_ps[tt][:, h * 65:(h + 1) * 65]
                if tt == 0:
                    nc.tensor.matmul(
                        dst, weiT[:, 0:P], Vtm[:, vo, h * 65:(h + 1) * 65],
                        start=True, stop=True)
                else:
                    nc.tensor.matmul(
                        dst, weiT[:, P:2 * P], Vtm[:, vo, h * 65:(h + 1) * 65],
                        start=True, stop=False)
                    nc.tensor.matmul(
                        dst, weiT[:, 2 * P:], Vtm[:, vo + 1, h * 65:(h + 1) * 65],
                        start=False, stop=True)

        def emit_norm(g, attn_ps, attn_sb, b2):
            """One reciprocal over the interleaved row sums, then normalize
            all heads during the PSUM->SBUF move (broadcast multiply)."""
            vo = b2 * 2
            for tt in range(2):
                aview = attn_ps[tt].rearrange("p (h e) -> p h e", e=65)
                rc6 = rc_pool.tile([P, H], F32, tag="rc", name=f"rc{g}{b2}{tt}")
                nc.vector.reciprocal(out=rc6, in_=aview[:, :, HS])
                rc_b = bass.AP(
                    tensor=rc6.tensor, offset=rc6.offset,
                    ap=[rc6.ap[0], rc6.ap[1], [0, HS]])
                nc.vector.tensor_tensor(
                    out=attn_sb[:, vo + tt, :].rearrange("p (h e) -> p h e", e=HS),
                    in0=aview[:, :, 0:HS], in1=rc_b, op=ALU.mult)

        def emit_wo_tt(g, attn_T, xa_t, xmid, tt):
            pso = ps_big.tile([P, C], F32, tag="big", name=f"pso{g}{tt}")
            for k in range(CK):
                nc.tensor.matmul(
                    pso, attn_T[:, k, tt * P:(tt + 1) * P], wo_sb[:, k, :],
                    start=(k == 0), stop=(not with_biases and k == CK - 1))
            if with_biases:
                nc.tensor.matmul(pso, ones1, bo_row, start=False, stop=True)
            nc.vector.tensor_add(out=xmid[:, tt, :], in0=xa_t[:, tt, :], in1=pso)

        def emit_ff1_third(g, h2T, ff1T, fs):
            for f in fs:
                psf = ps_mm.tile([P, ST], F32, tag="mm", name=f"psf{g}{f}")
                for k in range(CK):
                    nc.tensor.matmul(
                        psf, w1_sb[:, k, f * P:(f + 1) * P], h2T[:, k, :],
                        start=(k == 0), stop=(k == CK - 1))
                nc.scalar.activation(
                    out=ff1T[:, f, :], in_=psf, func=AF.Relu,
                    bias=(b1_sb[:, f:f + 1] if with_biases else 0.0))

        def emit_ff2(g, ff1T, xmid, t0):
            for tt in range(NTT):
                ps2 = ps_big.tile([P, C], F32, tag="big", name=f"ps2{g}{tt}")
                for f in range(FK):
                    nc.tensor.matmul(
                        ps2, ff1T[:, f, tt * P:(tt + 1) * P], w2_sb[:, f, :],
                        start=(f == 0), stop=(not with_biases and f == FK - 1))
                if with_biases:
                    nc.tensor.matmul(ps2, ones1, b2_row, start=False, stop=True)
                yt = io_pool.tile([P, C], F32, tag="yt", name=f"yt{g}{tt}")
                nc.vector.tensor_add(out=yt, in0=xmid[:, tt, :], in1=ps2)
                nc.sync.dma_start(
                    out=out[t0 + tt * P: t0 + (tt + 1) * P, :], in_=yt)

        # ------------------------------------------------------------------
        # supertile 0 prologue: interleave LN1(0) with QKV halves so the PE
        # starts as soon as the first token tiles land
        # ------------------------------------------------------------------
        qkt_tiles = {}
        vtm_tiles = {}
        h1T0 = act_pool.tile([P, CK, ST], BF16, tag="h1T", name="h1T0")
        for tt in (0, 1):
            hn = emit_ln_dve(xa0, tt, "a0")
            emit_tr(hn, h1T0, tt)
        emit_qkv(0, h1T0, half=0)
        emit_v(0, h1T0, (0, 1))
        for tt in (2, 3):
            hn = emit_ln_dve(xa0, tt, "a0")
            emit_tr(hn, h1T0, tt)
        emit_qkv(0, h1T0, half=1)
        emit_v(0, h1T0, (2, 3))
        h1T_cur = h1T0

        pend = None            # (h2T, xmid, t0) of supertile g-1
        for g in range(NST):
            t0 = g * ST
            xa_t = xa_tiles[g]

            if g + 1 < NST:
                xa_nxt = emit_xa_dma(g + 1)
                h1T_nxt = act_pool.tile(
                    [P, CK, ST], BF16, tag="h1T", name=f"h1T{g + 1}")
            if g > 0:
                # QKV/V for this supertile (prologue already did g=0)
                emit_qkv(g, h1T_cur)
                emit_v(g, h1T_cur, range(NTT))
            QT, KT = qkt_tiles[(g, "q")], qkt_tiles[(g, "k")]
            Vtm = vtm_tiles[g]

            if pend is not None:
                ff1T = act_pool.tile([P, FK, ST], BF16, tag="ff1T",
                                     name=f"ff1T{g - 1}")

            # ---- attention (2 batches) with ff1 filler, LN1(g+1) transposes
            # and the first attn transposes threaded between attnV heads ----
            attn_sb = hn_pool.tile([P, NTT, C], BF16, tag="attn_sb",
                                   name=f"asb{g}")
            attn_T = act_pool.tile([P, CK, ST], BF16, tag="attnT", name=f"aT{g}")
            for b2 in range(2):
                hns = {}
                if g + 1 < NST:
                    for tt in (2 * b2, 2 * b2 + 1):
                        hns[tt] = emit_ln_dve(xa_nxt, tt, f"a{g + 1}")
                weiTs = emit_scores(g, QT, KT, b2)
                attn_ps = [
                    ps_attn.tile([P, H * 65], F32, tag="attn",
                                 name=f"aps{g}_{b2}_{tt}")
                    for tt in range(2)
                ]
                if pend is not None:
                    emit_ff1_third(g, pend[0], ff1T, range(6 * b2, 6 * b2 + 3))
                for h in range(H):
                    emit_attnv_head(weiTs[h], Vtm, b2, attn_ps, h)
                    if h == 2:
                        if g + 1 < NST:
                            emit_tr(hns[2 * b2], h1T_nxt, 2 * b2)
                        if pend is not None:
                            emit_ff1_third(g, pend[0], ff1T,
                                           range(6 * b2 + 3, 6 * b2 + 6))
                        if b2 == 1:
                            emit_tr(attn_sb[:, 0, :], attn_T, 0)
                if g + 1 < NST:
                    emit_tr(hns[2 * b2 + 1], h1T_nxt, 2 * b2 + 1)
                emit_norm(g, attn_ps, attn_sb, b2)
                if b2 == 1:
                    emit_tr(attn_sb[:, 1, :], attn_T, 1)

            # ---- remaining attn transposes interleaved with Wo + residual ----
            xmid = io_pool.tile([P, NTT, C], F32, tag="xmid", name=f"xm{g}")
            emit_tr(attn_sb[:, 2, :], attn_T, 2)
            emit_wo_tt(g, attn_T, xa_t, xmid, 0)
            emit_tr(attn_sb[:, 3, :], attn_T, 3)
            for tt in range(1, NTT):
                emit_wo_tt(g, attn_T, xa_t, xmid, tt)

            # ---- LN2 DVE chains run under FF2(g-1)'s matmuls; the LN2
            # transposes are emitted after FF2 so they never block the PE ----
            h2T = act_pool.tile([P, CK, ST], BF16, tag="h2T", name=f"h2T{g}")
            hn2 = {tt: emit_ln_dve(xmid, tt, f"m{g}") for tt in range(NTT)}
            if pend is not None:
                emit_ff2(g, ff1T, pend[1], pend[2])
            for tt in range(NTT):
                emit_tr(hn2[tt], h2T, tt)

            pend = (h2T, xmid, t0)
            if g + 1 < NST:
                h1T_cur = h1T_nxt

        # epilogue: final supertile's FFN
        ff1T = act_pool.tile([P, FK, ST], BF16, tag="ff1T", name="ff1T7")
        emit_ff1_third(NST, pend[0], ff1T, range(0, 6))
        emit_ff1_third(NST, pend[0], ff1T, range(6, 12))
        emit_ff2(NST, ff1T, pend[1], pend[2])
    return nc


_NC_CACHE = {}


class _PinnedActBacc(bacc.Bacc):
    """Pin all ACT functions to the natural_log_exp_and_others table set.

    The kernel only uses Exp, Ln, Relu and Identity, all of which live in
    that one set; the default per-function greedy pick alternates between
    exp/sqrt/log sets and pays ~2.7us per switch. Blanking the other sets
    (indexes preserved) makes the fixpoint choose one set, loaded once.
    """

    def insert_act_table_loads(self):
        import concourse.mybir as _mb
        from concourse.hw_specs import get_activation_tables
        has_activation = any(
            isinstance(i, _mb.InstActivation)
            for b in self.main_func.blocks
            for i in b.instructions
        )
        if not has_activation:
            return
        keep = "natural_log_exp_and_others"
        tables = [
            (k, (v if k == keep else set()))
            for k, v in get_activation_tables(self.m.arch).items()
        ]
        bacc._bass_rust.insert_act_table_loads(self, tables)


def get_nc(with_biases=True):
    key = f"nc_b{int(with_biases)}"
    if key not in _NC_CACHE:
        nc = _PinnedActBacc(
            "TRN2", target_bir_lowering=False, debug=False, num_devices=NCORES)
        build_transformer(nc, with_biases=with_biases)
        nc.compile()
        _NC_CACHE[key] = nc
    return _NC_CACHE[key]


def prep_inputs(x, Wq, Wk, Wv, Wo, bo, W1, b1, W2, b2, g1, be1, g2, be2):
    """Host-side exact folding of LN affine params into weights/biases, plus
    layout packing and bf16 casts."""
    f32 = np.float32
    x = np.asarray(x, f32)
    Wq2 = np.asarray(Wq, f32).transpose(1, 0, 2).reshape(C, C)
    Wk2 = np.asarray(Wk, f32).transpose(1, 0, 2).reshape(C, C)
    Wv2 = np.asarray(Wv, f32).transpose(1, 0, 2).reshape(C, C)
    g1 = np.asarray(g1, f32)
    be1 = np.asarray(be1, f32)
    g2 = np.asarray(g2, f32)
    be2 = np.asarray(be2, f32)
    bf = lambda a: np.ascontiguousarray(np.asarray(a, f32)).astype(BF)
    shared = {
        "wq": bf((g1[:, None] * Wq2).reshape(CK, P, C).transpose(1, 0, 2)),
        "wk": bf((g1[:, None] * Wk2).reshape(CK, P, C).transpose(1, 0, 2)),
        "wv": bf((g1[:, None] * Wv2).reshape(CK, P, C).transpose(1, 0, 2)),
        "wo": bf(np.asarray(Wo, f32).reshape(CK, P, C).transpose(1, 0, 2)),
        "w1": bf((g2[:, None] * np.asarray(W1, f32)).reshape(CK, P, DFF).transpose(1, 0, 2)),
        "w2": bf(np.asarray(W2, f32).reshape(FK, P, C).transpose(1, 0, 2)),
    }
    biases = {
        "bq": np.ascontiguousarray(be1 @ Wq2),
        "bk": np.ascontiguousarray(be1 @ Wk2),
        "bv": bf(be1 @ Wv2),
        "bo": bf(np.asarray(bo, f32)),
        "b1": np.ascontiguousarray(be2 @ np.asarray(W1, f32) + np.asarray(b1, f32)),
        "b2": bf(np.asarray(b2, f32)),
    }
    shards = [
        np.ascontiguousarray(x[i * BL:(i + 1) * BL].reshape(NTOK, C))
        for i in range(NCORES)
    ]
    return shared, biases, shards


def all_biases_zero(biases):
    return all(
        not np.any(np.asarray(biases[k], np.float32))
        for k in ("bq", "bk", "bv", "bo", "b1", "b2"))


def run_on_device(nc, shared, shards, trace=False, **kwargs):
    in_maps = [dict(shared, xs=shards[i]) for i in range(NCORES)]
    return run_bass_kernel_spmd(
        nc, in_maps, core_ids=list(range(NCORES)), trace=trace, **kwargs)


def kernel(**inputs):
    shared, biases, shards = prep_inputs(**inputs)
    wb = not all_biases_zero(biases)
    if wb:
        shared = dict(shared, **biases)
    nc = get_nc(with_biases=wb)
    res = run_on_device(nc, shared, shards, trace=False)
    out = np.concatenate(
        [res.results[i]["out"].reshape(BL, T, C) for i in range(NCORES)], axis=0)
    return out.astype(np.float32)
